# revision 8
# baseline (speedup 1.0000x reference)
"""SMPL BodyModel Trainium2 kernel.

Sharding: 8 cores = 4 batch-groups (128 batches) x 2 vertex-halves (3445
vertices). Joint-space quantities (rodrigues, kinematic chain, A) are
computed redundantly on both cores of a batch-group; vertex-space work
(shape/pose blendshapes, skinning) is split by vertex half.

Per-core pipeline (batch-major: 128 batches on partitions):
  - JSD trick: J_regressor-contracted shapedirs/v_template (contraction
    over full V on PE) -> joint locations directly from betas.
  - Rodrigues vectorized over 24 joints as (128, 24) DVE/ACT ops.
  - Kinematic chain: per-joint lane-wise 3x4 @ 4x4 on DVE.
  - v_posed = [betas|1] @ [SD^T|vt] + pf^T @ posedirs (PSUM accumulated,
    streamed by 512-column tiles).
  - Skinning: T_pk = A^T-slice (stationary) @ lbs^T (moving);
    verts = sum_k T_pk * vph_k element-wise on DVE/GPSIMD.
All layout changes use packed PE transposes + small SBUF->SBUF DMAs
(f32 has no DMA-transpose path).
"""

from contextlib import ExitStack

import numpy as np

B, V, J, NB = 512, 6890, 24, 10
P = (J - 1) * 9  # 207
NCORES = 8
BG = 128          # batches per group
VH = V // 2       # 3445 vertices per half
VKH = VH * 3      # 10335
NT = 512          # N tile for streamed matmuls

_CACHE = {}
LAST_RESULTS = None


def _parent_runs(par):
    """Contiguous (j0, count, p0, pstep) runs of j>=1 with affine parents."""
    runs = []
    j, n = 1, len(par)
    while j < n:
        j0, p0 = j, int(par[j])
        pstep = (int(par[j + 1]) - p0) if j + 1 < n else 0
        if pstep not in (0, 1):
            pstep = 0
        cnt = 1
        while j0 + cnt < n and int(par[j0 + cnt]) == p0 + pstep * cnt:
            cnt += 1
        if cnt == 1:
            pstep = 0
        runs.append((j0, cnt, p0, pstep))
        j = j0 + cnt
    return runs


def _cdiv(a, b):
    return (a + b - 1) // b


def _build(par):
    import concourse.bass as bass
    import concourse.tile as tile
    from concourse import bacc, mybir
    from concourse.masks import make_identity

    f32 = mybir.dt.float32
    Alu = mybir.AluOpType
    Act = mybir.ActivationFunctionType

    nc = bacc.Bacc("TRN2", target_bir_lowering=False, debug=False)

    betas_d = nc.dram_tensor("betas_g", [BG, NB], f32, kind="ExternalInput").ap()
    pose_d = nc.dram_tensor("pose_g", [BG, J * 3], f32, kind="ExternalInput").ap()
    pd_d = nc.dram_tensor("pd_half", [P, VKH], f32, kind="ExternalInput").ap()
    sdh_d = nc.dram_tensor("sd_half", [VKH, NB], f32, kind="ExternalInput").ap()
    sdf_d = nc.dram_tensor("sd_full", [V, 3 * NB], f32, kind="ExternalInput").ap()
    vth_d = nc.dram_tensor("vt_half", [VKH], f32, kind="ExternalInput").ap()
    vtf_d = nc.dram_tensor("vt_full", [V, 3], f32, kind="ExternalInput").ap()
    jreg_d = nc.dram_tensor("jreg", [J, V], f32, kind="ExternalInput").ap()
    lbs_d = nc.dram_tensor("lbs_half", [VH, J], f32, kind="ExternalInput").ap()

    verts_o = nc.dram_tensor("verts", [BG, VH, 3], f32, kind="ExternalOutput").ap()
    vposed_o = nc.dram_tensor("vposed", [BG, VH, 3], f32, kind="ExternalOutput").ap()
    A_o = nc.dram_tensor("A_out", [BG, J, 4, 4], f32, kind="ExternalOutput").ap()
    tf_o = nc.dram_tensor("tf_out", [BG, J, 4, 4], f32, kind="ExternalOutput").ap()
    pj_o = nc.dram_tensor("pj_out", [BG, J, 3], f32, kind="ExternalOutput").ap()

    NVC = _cdiv(V, 128)           # 54 chunks over full V (contraction)
    VT = NVC * 128 - V            # 22 pad rows in last chunk
    NKC = _cdiv(VKH, 128)         # 81 chunks over half vk rows
    KT = NKC * 128 - VKH          # 33
    NLC = _cdiv(VH, 128)          # 27 chunks over half v rows
    LT = NLC * 128 - VH           # 11
    NTN = _cdiv(VKH, NT)          # 21 streamed N tiles over (v,k)
    NTV = _cdiv(VH, NT)           # 7 v tiles for skinning
    PK = 12                       # sd chunks packed per PE transpose
    NGK = _cdiv(NKC, PK)          # 7
    PL = 5                        # lbs chunks packed per PE transpose
    NGL = _cdiv(NLC, PL)          # 6

    def bcast_j(ap3, p0, cnt):
        """(p, 3, 1) column p0 broadcast to (p, 3, cnt) via 0-stride dim."""
        src = ap3[:, :, p0:p0 + 1]
        return bass.AP(tensor=src.tensor, offset=src.offset,
                       ap=list(src.ap)[:2] + [[0, cnt]])

    with tile.TileContext(nc) as tc, ExitStack() as st:
        # ---------------- persistent pools ----------------
        big = st.enter_context(tc.tile_pool(name="big", bufs=1))
        sm = st.enter_context(tc.tile_pool(name="sm", bufs=1))

        sdt = big.tile([NB + 1, NGK * PK * 128], f32)    # [SD^T | vt] rows
        lbsT = big.tile([J, NGL * PL * 128], f32)
        vp_sb = big.tile([BG, VKH], f32)

        ident = sm.tile([128, 128], f32)
        make_identity(nc, ident)

        betas_sb = sm.tile([BG, NB], f32)
        nc.sync.dma_start(out=betas_sb, in_=betas_d)
        pose_sb = sm.tile([BG, J * 3], f32)
        nc.sync.dma_start(out=pose_sb, in_=pose_d)

        with tc.tile_pool(name="ph1", bufs=1) as ph1, \
             tc.tile_pool(name="ps1", bufs=2, space="PSUM") as ps1, \
             tc.tile_pool(name="psjl", bufs=1, space="PSUM") as psjl, \
             tc.tile_pool(name="psacc", bufs=1, space="PSUM") as psacc:

            # ===== raw loads =====
            jreg_sb = ph1.tile([J, V], f32)
            nc.sync.dma_start(out=jreg_sb, in_=jreg_d)

            sdf_sb = ph1.tile([128, NVC, 3 * NB], f32)
            nc.sync.dma_start(
                out=sdf_sb[:, : NVC - 1, :],
                in_=sdf_d[: 128 * (NVC - 1), :].rearrange("(c p) e -> p c e", p=128))
            nc.sync.dma_start(out=sdf_sb[: 128 - VT, NVC - 1, :],
                              in_=sdf_d[128 * (NVC - 1):, :])
            vtf_sb = ph1.tile([128, NVC, 3], f32)
            nc.sync.dma_start(
                out=vtf_sb[:, : NVC - 1, :],
                in_=vtf_d[: 128 * (NVC - 1), :].rearrange("(c p) e -> p c e", p=128))
            nc.sync.dma_start(out=vtf_sb[: 128 - VT, NVC - 1, :],
                              in_=vtf_d[128 * (NVC - 1):, :])

            sdh_sb = ph1.tile([128, NKC, NB], f32)
            nc.vector.memset(sdh_sb[:, NKC - 1, :], 0.0)
            nc.sync.dma_start(
                out=sdh_sb[:, : NKC - 1, :],
                in_=sdh_d[: 128 * (NKC - 1), :].rearrange("(c p) e -> p c e", p=128))
            nc.sync.dma_start(out=sdh_sb[: 128 - KT, NKC - 1, :],
                              in_=sdh_d[128 * (NKC - 1):, :])

            lbsh_sb = ph1.tile([128, NLC, J], f32)
            nc.vector.memset(lbsh_sb[:, NLC - 1, :], 0.0)
            nc.sync.dma_start(
                out=lbsh_sb[:, : NLC - 1, :],
                in_=lbs_d[: 128 * (NLC - 1), :].rearrange("(c p) e -> p c e", p=128))
            nc.sync.dma_start(out=lbsh_sb[: 128 - LT, NLC - 1, :],
                              in_=lbs_d[128 * (NLC - 1):, :])

            # ===== JregT via packed PE transposes =====
            jregT = ph1.tile([128, NVC * J], f32)        # (v128, c*24+j)
            for g in range(_cdiv(NVC, 4)):
                cs = list(range(g * 4, min(g * 4 + 4, NVC)))
                ps = ps1.tile([128, 4 * J], f32, tag="ps_tr")
                for i, c in enumerate(cs):
                    w = 128 if c < NVC - 1 else 128 - VT
                    nc.tensor.transpose(ps[:w, i * J:(i + 1) * J],
                                        jreg_sb[:, c * 128: c * 128 + w],
                                        ident[:J, :J])
                nc.scalar.copy(out=jregT[:, g * 4 * J: (g * 4 + len(cs)) * J],
                               in_=ps[:, : len(cs) * J])

            # ===== JSD accumulation over full V =====
            jsd_ps = psacc.tile([J, 3 * NB], f32, tag="jsd")
            jvt_ps = psacc.tile([J, 3], f32, tag="jvt")
            for c in range(NVC):
                w = 128 if c < NVC - 1 else 128 - VT
                kw = dict(start=(c == 0), stop=(c == NVC - 1))
                lt = jregT[:w, c * J:(c + 1) * J]
                nc.tensor.matmul(jsd_ps, lt, sdf_sb[:w, c, :], **kw)
                nc.tensor.matmul(jvt_ps, lt, vtf_sb[:w, c, :], **kw)
            jsd_sb = sm.tile([J, 3 * NB], f32)
            nc.scalar.copy(out=jsd_sb, in_=jsd_ps)
            jvt_sb = sm.tile([J, 3], f32)
            nc.scalar.copy(out=jvt_sb, in_=jvt_ps)

            # jsdr (11, (k,j)): rows 0..9 JSD_l, row 10 Jvt
            jsdr = sm.tile([NB + 1, 3 * J], f32)
            ps_j = ps1.tile([NB, 3 * J], f32, tag="ps_tr")
            for k in range(3):
                nc.tensor.transpose(ps_j[:, k * J:(k + 1) * J],
                                    jsd_sb[:, k * NB:(k + 1) * NB], ident[:J, :J])
            nc.scalar.copy(out=jsdr[:NB, :], in_=ps_j)
            ps_v = ps1.tile([3, J], f32, tag="ps_tr2")
            nc.tensor.transpose(ps_v, jvt_sb, ident[:J, :J])
            jvtT = sm.tile([3, J], f32)
            nc.scalar.copy(out=jvtT, in_=ps_v)
            for k in range(3):
                nc.sync.dma_start(out=jsdr[NB:NB + 1, k * J:(k + 1) * J],
                                  in_=jvtT[k:k + 1, :])

            # ===== betasT (+ones row), Jloc =====
            betasT = sm.tile([NB + 1, BG], f32)
            nc.vector.memset(betasT, 1.0)
            ps_b = ps1.tile([NB, BG], f32, tag="ps_tr")
            nc.tensor.transpose(ps_b, betas_sb, ident[:BG, :BG])
            nc.scalar.copy(out=betasT[:NB, :], in_=ps_b)

            jloc_ps = psjl.tile([BG, 3 * J], f32, tag="ps_jl")
            nc.tensor.matmul(jloc_ps, betasT, jsdr, start=True, stop=True)
            jloc = sm.tile([BG, 3 * J], f32)
            nc.scalar.copy(out=jloc, in_=jloc_ps)

            # ===== Rodrigues =====
            pose3 = pose_sb.rearrange("p (j k) -> p k j", k=3)
            bias0 = sm.tile([BG, 1], f32)
            nc.vector.memset(bias0, 0.0)
            biasq = sm.tile([BG, 1], f32)
            nc.vector.memset(biasq, float(np.pi / 2))
            peps = sm.tile([BG, J * 3], f32)
            nc.vector.tensor_scalar_add(peps, pose_sb, 1e-8)
            sq = sm.tile([BG, J * 3], f32)
            nc.vector.tensor_mul(sq, peps, peps)
            sq3 = sq.rearrange("p (j k) -> p k j", k=3)
            ang = sm.tile([BG, J], f32)
            nc.vector.tensor_add(ang, sq3[:, 0, :], sq3[:, 1, :])
            nc.vector.tensor_add(ang, ang, sq3[:, 2, :])
            nc.scalar.activation(out=ang, in_=ang, func=Act.Sqrt, bias=bias0)
            inv = sm.tile([BG, J], f32)
            nc.vector.reciprocal(out=inv, in_=ang)
            sin = sm.tile([BG, J], f32)
            nc.scalar.activation(out=sin, in_=ang, func=Act.Sin, bias=bias0)
            omc = sm.tile([BG, J], f32)          # 1 - cos(ang)
            nc.scalar.activation(out=omc, in_=ang, func=Act.Sin, bias=biasq)
            nc.vector.tensor_scalar(omc, omc, -1.0, 1.0, Alu.mult, Alu.add)

            rd = sm.tile([BG, 3, J], f32)
            for k in range(3):
                nc.vector.tensor_mul(rd[:, k, :], pose3[:, k, :], inv)
            pr = sm.tile([BG, 9, J], f32)   # xx yy zz xy xz yz sx sy sz
            for i, (a, b) in enumerate([(0, 0), (1, 1), (2, 2),
                                        (0, 1), (0, 2), (1, 2)]):
                nc.vector.tensor_mul(pr[:, i, :], rd[:, a, :], rd[:, b, :])
            for k in range(3):
                nc.vector.tensor_mul(pr[:, 6 + k, :], sin, rd[:, k, :])
            XX, YY, ZZ, XY, XZ, YZ, SX, SY, SZ = range(9)

            R = sm.tile([BG, 9, J], f32)    # (r*3+c, j), diag includes +1
            t9 = sm.tile([BG, 3, J], f32)   # omc*(sum of other squares)
            for r, (a, b) in enumerate([(YY, ZZ), (XX, ZZ), (XX, YY)]):
                nc.vector.tensor_add(t9[:, r, :], pr[:, a, :], pr[:, b, :])
                nc.vector.tensor_mul(t9[:, r, :], t9[:, r, :], omc)
                nc.vector.tensor_scalar(R[:, r * 3 + r, :], t9[:, r, :],
                                        -1.0, 1.0, Alu.mult, Alu.add)
            m9 = sm.tile([BG, 3, J], f32)
            for i, pidx in enumerate([XY, XZ, YZ]):
                nc.vector.tensor_mul(m9[:, i, :], pr[:, pidx, :], omc)
            for (rc, mi, si, op) in [(1, 0, SZ, Alu.subtract), (3, 0, SZ, Alu.add),
                                     (2, 1, SY, Alu.add), (6, 1, SY, Alu.subtract),
                                     (5, 2, SX, Alu.subtract), (7, 2, SX, Alu.add)]:
                nc.vector.tensor_tensor(R[:, rc, :], m9[:, mi, :], pr[:, si, :], op)

            # ===== pose_feature -> pfT =====
            pf = sm.tile([BG, J - 1, 9], f32)
            for rc in range(9):
                if rc in (0, 4, 8):
                    nc.vector.tensor_scalar_mul(pf[:, :, rc],
                                                t9[:, rc // 3, 1:], -1.0)
                else:
                    nc.vector.tensor_copy(out=pf[:, :, rc], in_=R[:, rc, 1:])
            pff = pf.rearrange("p a b -> p (a b)")
            pfT0 = sm.tile([128, BG], f32)
            ps_p = ps1.tile([128, BG], f32, tag="ps_tr")
            nc.tensor.transpose(ps_p, pff[:, :128], ident)
            nc.scalar.copy(out=pfT0, in_=ps_p)
            pfT1 = sm.tile([P - 128, BG], f32)
            ps_q = ps1.tile([P - 128, BG], f32, tag="ps_tr2")
            nc.tensor.transpose(ps_q, pff[:, 128:P], ident)
            nc.scalar.copy(out=pfT1, in_=ps_q)

            # ===== rel, Tm, kinematic chain =====
            j3 = jloc.rearrange("p (k j) -> p k j", k=3)
            rel = sm.tile([BG, 3, J], f32)
            nc.vector.tensor_copy(out=rel[:, :, 0], in_=j3[:, :, 0])
            for (j0, cnt, p0, pstep) in _parent_runs(par):
                if pstep == 1:
                    src = j3[:, :, p0:p0 + cnt]
                else:
                    src = bcast_j(j3, p0, cnt)
                nc.vector.tensor_sub(rel[:, :, j0:j0 + cnt],
                                     j3[:, :, j0:j0 + cnt], src)

            Tm = sm.tile([BG, J, 12], f32)
            Tm4 = Tm.rearrange("p j (r c) -> p j r c", c=4)
            nc.vector.tensor_copy(out=Tm4[:, :, :, :3],
                                  in_=R.rearrange("p (r c) j -> p j r c", c=3))
            nc.vector.tensor_copy(out=Tm4[:, :, :, 3],
                                  in_=rel.rearrange("p k j -> p j k"))

            W = sm.tile([BG, J, 12], f32)
            W4 = W.rearrange("p j (r c) -> p j r c", c=4)
            nc.vector.tensor_copy(out=W[:, 0, :], in_=Tm[:, 0, :])
            for j in range(1, J):
                pj = int(par[j])
                for r in range(3):
                    nc.vector.tensor_scalar_mul(W4[:, j, r, :], Tm4[:, j, 0, :],
                                                W4[:, pj, r, 0:1])
                    for m in (1, 2):
                        nc.vector.scalar_tensor_tensor(
                            W4[:, j, r, :], Tm4[:, j, m, :], W4[:, pj, r, m:m + 1],
                            W4[:, j, r, :], Alu.mult, Alu.add)
                    nc.vector.tensor_scalar_add(W4[:, j, r, 3:4], W4[:, j, r, 3:4],
                                                W4[:, pj, r, 3:4])

            # transforms / posed_joints outputs
            nc.sync.dma_start(out=tf_o[:, :, :3, :],
                              in_=W.rearrange("p j (r c) -> p j r c", c=4))
            bot = sm.tile([BG, J, 4], f32)
            nc.vector.memset(bot, 0.0)
            nc.vector.memset(bot[:, :, 3], 1.0)
            nc.sync.dma_start(out=tf_o[:, :, 3, :], in_=bot)
            pjc = sm.tile([BG, J, 3], f32)
            nc.vector.tensor_copy(out=pjc, in_=W4[:, :, :, 3])
            nc.sync.dma_start(out=pj_o, in_=pjc)

            # ===== A = W - [0 | W @ [Jloc;0]] (gpsimd) =====
            Am = sm.tile([BG, J, 12], f32)
            Am4 = Am.rearrange("p j (r c) -> p j r c", c=4)
            nc.gpsimd.tensor_copy(out=Am4[:, :, :, :3], in_=W4[:, :, :, :3])
            acc3 = sm.tile([BG, 3, J], f32)
            t3 = sm.tile([BG, J], f32)
            for r in range(3):
                nc.gpsimd.tensor_mul(acc3[:, r, :], W4[:, :, r, 0], j3[:, 0, :])
                for k in (1, 2):
                    nc.gpsimd.tensor_mul(t3, W4[:, :, r, k], j3[:, k, :])
                    nc.gpsimd.tensor_add(acc3[:, r, :], acc3[:, r, :], t3)
                nc.gpsimd.tensor_sub(Am4[:, :, r, 3], W4[:, :, r, 3], acc3[:, r, :])
            nc.sync.dma_start(out=A_o[:, :, :3, :],
                              in_=Am.rearrange("p j (r c) -> p j r c", c=4))
            nc.sync.dma_start(out=A_o[:, :, 3, :], in_=bot)

            # ===== A_T (24, 12, 128) =====
            AT = sm.tile([J, 12, BG], f32)
            for g in range(3):
                ps_a = ps1.tile([J, 4 * BG], f32, tag="ps_tr")
                for i in range(4):
                    nc.tensor.transpose(ps_a[:, i * BG:(i + 1) * BG],
                                        Am[:, :, g * 4 + i], ident[:BG, :BG])
                nc.scalar.copy(
                    out=AT[:, g * 4:(g + 1) * 4, :].rearrange("j a b -> j (a b)"),
                    in_=ps_a)

            # ===== SDT build =====
            sdt_stage = ph1.tile([PK * NB, NGK * 128], f32)
            sdh_f = sdh_sb.rearrange("p c e -> p (c e)")
            for g in range(NGK):
                c0 = g * PK
                nch = min(PK, NKC - c0)
                ps_s = ps1.tile([PK * NB, 128], f32, tag="ps_tr")
                nc.tensor.transpose(ps_s[: nch * NB, :],
                                    sdh_f[:, c0 * NB:(c0 + nch) * NB], ident)
                nc.scalar.copy(out=sdt_stage[: nch * NB, g * 128:(g + 1) * 128],
                               in_=ps_s[: nch * NB, :])
            sdt4 = sdt.rearrange("l (g c p) -> l c g p", c=PK, p=128)
            for c in range(PK):
                gcnt = len([g for g in range(NGK) if g * PK + c < NKC])
                nc.sync.dma_start(
                    out=sdt4[:NB, c, :gcnt, :],
                    in_=sdt_stage[c * NB:(c + 1) * NB, : gcnt * 128]
                        .rearrange("l (g p) -> l g p", p=128))
            nc.sync.dma_start(out=sdt[NB:NB + 1, :VKH], in_=vth_d[None, :])

            # ===== lbsT build =====
            lbst_stage = ph1.tile([PL * J, NGL * 128], f32)
            lbsh_f = lbsh_sb.rearrange("p c e -> p (c e)")
            for g in range(NGL):
                c0 = g * PL
                ncl = min(PL, NLC - c0)
                ps_l = ps1.tile([PL * J, 128], f32, tag="ps_tr")
                nc.tensor.transpose(ps_l[: ncl * J, :],
                                    lbsh_f[:, c0 * J:(c0 + ncl) * J], ident)
                nc.scalar.copy(out=lbst_stage[: ncl * J, g * 128:(g + 1) * 128],
                               in_=ps_l[: ncl * J, :])
            lbsT4 = lbsT.rearrange("j (g c p) -> j c g p", c=PL, p=128)
            for c in range(PL):
                gcnt = len([g for g in range(NGL) if g * PL + c < NLC])
                nc.sync.dma_start(
                    out=lbsT4[:, c, :gcnt, :],
                    in_=lbst_stage[c * J:(c + 1) * J, : gcnt * 128]
                        .rearrange("j (g p) -> j g p", p=128))

        # ---------------- v_posed stream ----------------
        with tc.tile_pool(name="pdst", bufs=3) as pdst, \
             tc.tile_pool(name="psvp", bufs=2, space="PSUM") as psvp:
            for t in range(NTN):
                n0 = t * NT
                nn = min(NT, VKH - n0)
                pd0 = pdst.tile([128, NT], f32, tag="pd0")
                nc.sync.dma_start(out=pd0[:, :nn], in_=pd_d[:128, n0:n0 + nn])
                pd1 = pdst.tile([P - 128, NT], f32, tag="pd1")
                nc.sync.dma_start(out=pd1[:, :nn], in_=pd_d[128:, n0:n0 + nn])
                ps_vp = psvp.tile([BG, NT], f32, tag="ps_vp")
                nc.tensor.matmul(ps_vp[:, :nn], betasT, sdt[:, n0:n0 + nn],
                                 start=True, stop=False)
                nc.tensor.matmul(ps_vp[:, :nn], pfT0, pd0[:, :nn],
                                 start=False, stop=False)
                nc.tensor.matmul(ps_vp[:, :nn], pfT1, pd1[:, :nn],
                                 start=False, stop=True)
                nc.scalar.copy(out=vp_sb[:, n0:n0 + nn], in_=ps_vp[:, :nn])
                nc.sync.dma_start(
                    out=vposed_o.rearrange("b v k -> b (v k)")[:, n0:n0 + nn],
                    in_=vp_sb[:, n0:n0 + nn])

        # ---------------- skinning ----------------
        vp3 = vp_sb.rearrange("p (v k) -> p k v", k=3)
        with tc.tile_pool(name="vout", bufs=2) as vout, \
             tc.tile_pool(name="mt", bufs=2) as mt, \
             tc.tile_pool(name="psT", bufs=2, space="PSUM") as psT:
            for t in range(NTV):
                v0 = t * NT
                nv = min(NT, VH - v0)
                vo = vout.tile([BG, NT, 3], f32, tag="vo")
                for p in range(3):
                    ps_T = psT.tile([BG, 4, NT], f32, tag="ps_T")
                    for k in range(4):
                        nc.tensor.matmul(ps_T[:, k, :nv], AT[:, p * 4 + k, :],
                                         lbsT[:, v0:v0 + nv], start=True, stop=True)
                    m0 = mt.tile([BG, NT], f32, tag="m0")
                    m1 = mt.tile([BG, NT], f32, tag="m1")
                    t3sb = mt.tile([BG, NT], f32, tag="t3sb")
                    nc.vector.tensor_mul(m0[:, :nv], ps_T[:, 0, :nv],
                                         vp3[:, 0, v0:v0 + nv])
                    nc.vector.tensor_mul(m1[:, :nv], ps_T[:, 1, :nv],
                                         vp3[:, 1, v0:v0 + nv])
                    nc.scalar.copy(out=t3sb[:, :nv], in_=ps_T[:, 3, :nv])
                    nc.gpsimd.tensor_add(m0[:, :nv], m0[:, :nv], m1[:, :nv])
                    nc.vector.tensor_mul(m1[:, :nv], ps_T[:, 2, :nv],
                                         vp3[:, 2, v0:v0 + nv])
                    nc.gpsimd.tensor_add(m0[:, :nv], m0[:, :nv], t3sb[:, :nv])
                    nc.vector.tensor_add(vo[:, :nv, p], m0[:, :nv], m1[:, :nv])
                nc.sync.dma_start(out=verts_o[:, v0:v0 + nv, :],
                                  in_=vo[:, :nv, :])

    nc.compile()
    return nc


def _get_nc(par_key):
    if par_key not in _CACHE:
        _CACHE[par_key] = _build(np.asarray(par_key, dtype=np.int32))
    return _CACHE[par_key]


def kernel(betas, pose, v_template, shapedirs, posedirs, J_regressor,
           lbs_weights, parents):
    from concourse.bass_utils import run_bass_kernel_spmd

    par = tuple(int(x) for x in np.asarray(parents))
    nc = _get_nc(par)

    f = np.ascontiguousarray
    sd_flat = np.asarray(shapedirs, np.float32).reshape(V * 3, NB)
    sd_full = np.asarray(shapedirs, np.float32).reshape(V, 3 * NB)
    vt_flat = np.asarray(v_template, np.float32).reshape(V * 3)
    pd = np.asarray(posedirs, np.float32)
    jreg = np.asarray(J_regressor, np.float32)
    lbs = np.asarray(lbs_weights, np.float32)

    in_maps = []
    for c in range(NCORES):
        bg, vh = c // 2, c % 2
        in_maps.append({
            "betas_g": f(np.asarray(betas, np.float32)[bg * BG:(bg + 1) * BG]),
            "pose_g": f(np.asarray(pose, np.float32)[bg * BG:(bg + 1) * BG]),
            "pd_half": f(pd[:, vh * VKH:(vh + 1) * VKH]),
            "sd_half": f(sd_flat[vh * VKH:(vh + 1) * VKH]),
            "sd_full": f(sd_full),
            "vt_half": f(vt_flat[vh * VKH:(vh + 1) * VKH]),
            "vt_full": f(np.asarray(v_template, np.float32)),
            "jreg": f(jreg),
            "lbs_half": f(lbs[vh * VH:(vh + 1) * VH]),
        })

    import os as _os
    _r = run_bass_kernel_spmd(
        nc, in_maps, core_ids=list(range(NCORES)),
        trace=bool(_os.environ.get("BM_TRACE")))
    global LAST_RESULTS
    LAST_RESULTS = _r
    res = _r.results

    verts = np.empty((B, V, 3), np.float32)
    vposed = np.empty((B, V, 3), np.float32)
    A = np.empty((B, J, 4, 4), np.float32)
    tf = np.empty((B, J, 4, 4), np.float32)
    pj = np.empty((B, J, 3), np.float32)
    for c in range(NCORES):
        bg, vh = c // 2, c % 2
        bs = slice(bg * BG, (bg + 1) * BG)
        vs = slice(vh * VH, (vh + 1) * VH)
        verts[bs, vs] = res[c]["verts"]
        vposed[bs, vs] = res[c]["vposed"]
        if vh == 0:
            A[bs] = res[c]["A_out"]
            tf[bs] = res[c]["tf_out"]
            pj[bs] = res[c]["pj_out"]
    return verts, pj, A, tf, vposed


# revision 10
# speedup vs baseline: 1.4883x; 1.4883x over previous
"""SMPL BodyModel Trainium2 kernel.

Sharding: 8 cores = 4 batch-groups (128 batches) x 2 vertex-halves (3445
vertices). Joint-space quantities (rodrigues, kinematic chain, A) are
computed redundantly on both cores of a batch-group; vertex-space work
(shape/pose blendshapes, skinning) is split by vertex half.

Per-core pipeline (batch-major: 128 batches on partitions):
  - JSD trick: J_regressor-contracted shapedirs/v_template (contraction
    over full V on PE) -> joint locations directly from betas.
  - Rodrigues vectorized over 24 joints as (128, 24) DVE/ACT ops.
  - Kinematic chain: per-joint lane-wise 3x4 @ 4x4 on DVE.
  - v_posed = [betas|1] @ [SD^T|vt] + pf^T @ posedirs (PSUM accumulated,
    streamed by 512-column tiles).
  - Skinning: T_pk = A^T-slice (stationary) @ lbs^T (moving);
    verts = sum_k T_pk * vph_k element-wise on DVE/GPSIMD.
All layout changes use packed PE transposes + small SBUF->SBUF DMAs
(f32 has no DMA-transpose path).
"""

from contextlib import ExitStack

import numpy as np

B, V, J, NB = 512, 6890, 24, 10
P = (J - 1) * 9  # 207
NCORES = 8
BG = 128          # batches per group
VH = V // 2       # 3445 vertices per half
VKH = VH * 3      # 10335
NT = 512          # N tile for streamed matmuls

_CACHE = {}
LAST_RESULTS = None


def _parent_runs(par):
    """Contiguous (j0, count, p0, pstep) runs of j>=1 with affine parents."""
    runs = []
    j, n = 1, len(par)
    while j < n:
        j0, p0 = j, int(par[j])
        pstep = (int(par[j + 1]) - p0) if j + 1 < n else 0
        if pstep not in (0, 1):
            pstep = 0
        cnt = 1
        while j0 + cnt < n and int(par[j0 + cnt]) == p0 + pstep * cnt:
            cnt += 1
        if cnt == 1:
            pstep = 0
        runs.append((j0, cnt, p0, pstep))
        j = j0 + cnt
    return runs


def _cdiv(a, b):
    return (a + b - 1) // b


def _build(par):
    import concourse.bass as bass
    import concourse.tile as tile
    from concourse import bacc, mybir
    from concourse.masks import make_identity

    f32 = mybir.dt.float32
    bf16 = mybir.dt.bfloat16
    Alu = mybir.AluOpType
    Act = mybir.ActivationFunctionType

    nc = bacc.Bacc("TRN2", target_bir_lowering=False, debug=False)

    betas_d = nc.dram_tensor("betas_g", [BG, NB], f32, kind="ExternalInput").ap()
    pose_d = nc.dram_tensor("pose_g", [BG, J * 3], f32, kind="ExternalInput").ap()
    pd_d = nc.dram_tensor("pd_half", [P, VKH], bf16, kind="ExternalInput").ap()
    sdh_d = nc.dram_tensor("sd_half", [VKH, NB], f32, kind="ExternalInput").ap()
    sdf_d = nc.dram_tensor("sd_full", [V, 3 * NB], f32, kind="ExternalInput").ap()
    vth_d = nc.dram_tensor("vt_half", [VKH], f32, kind="ExternalInput").ap()
    vtf_d = nc.dram_tensor("vt_full", [V, 3], f32, kind="ExternalInput").ap()
    jreg_d = nc.dram_tensor("jreg", [J, V], f32, kind="ExternalInput").ap()
    lbs_d = nc.dram_tensor("lbs_half", [VH, J], f32, kind="ExternalInput").ap()

    verts_o = nc.dram_tensor("verts", [BG, VH, 3], f32, kind="ExternalOutput").ap()
    vposed_o = nc.dram_tensor("vposed", [BG, VH, 3], f32, kind="ExternalOutput").ap()
    A_o = nc.dram_tensor("A_out", [BG, J, 4, 4], f32, kind="ExternalOutput").ap()
    tf_o = nc.dram_tensor("tf_out", [BG, J, 4, 4], f32, kind="ExternalOutput").ap()
    pj_o = nc.dram_tensor("pj_out", [BG, J, 3], f32, kind="ExternalOutput").ap()

    NVC = _cdiv(V, 128)           # 54 chunks over full V (contraction)
    VT = NVC * 128 - V            # 22 pad rows in last chunk
    NKC = _cdiv(VKH, 128)         # 81 chunks over half vk rows
    KT = NKC * 128 - VKH          # 33
    NLC = _cdiv(VH, 128)          # 27 chunks over half v rows
    LT = NLC * 128 - VH           # 11
    NTN = _cdiv(VKH, NT)          # 21 streamed N tiles over (v,k)
    NTV = _cdiv(VH, NT)           # 7 v tiles for skinning
    PK = 12                       # sd chunks packed per PE transpose
    NGK = _cdiv(NKC, PK)          # 7
    PL = 5                        # lbs chunks packed per PE transpose
    NGL = _cdiv(NLC, PL)          # 6

    def bcast_j(ap3, p0, cnt):
        """(p, 3, 1) column p0 broadcast to (p, 3, cnt) via 0-stride dim."""
        src = ap3[:, :, p0:p0 + 1]
        return bass.AP(tensor=src.tensor, offset=src.offset,
                       ap=list(src.ap)[:2] + [[0, cnt]])

    with tile.TileContext(nc) as tc, ExitStack() as st:
        # ---------------- persistent pools ----------------
        big = st.enter_context(tc.tile_pool(name="big", bufs=1))
        sm = st.enter_context(tc.tile_pool(name="sm", bufs=1))

        sdt = big.tile([NB + 1, NGK * PK * 128], f32)    # [SD^T | vt] rows
        lbsT = big.tile([J, NGL * PL * 128], bf16)
        vp_sb = big.tile([BG, VKH], f32)

        AT = sm.tile([J, 12, BG], bf16)
        ident = sm.tile([128, 128], f32)
        make_identity(nc, ident)

        betas_sb = sm.tile([BG, NB], f32)
        nc.sync.dma_start(out=betas_sb, in_=betas_d)
        pose_sb = sm.tile([BG, J * 3], f32)
        nc.sync.dma_start(out=pose_sb, in_=pose_d)

        with tc.tile_pool(name="ph1", bufs=1) as ph1, \
             tc.tile_pool(name="ps1", bufs=2, space="PSUM") as ps1, \
             tc.tile_pool(name="psjl", bufs=1, space="PSUM") as psjl, \
             tc.tile_pool(name="psacc", bufs=1, space="PSUM") as psacc:

            # ===== raw loads (jreg+sdh first: they gate PE start) =====
            jreg_sb = ph1.tile([J, V], f32)
            nc.sync.dma_start(out=jreg_sb, in_=jreg_d)

            sdh_sb = ph1.tile([128, NKC, NB], f32)
            nc.vector.memset(sdh_sb[:, NKC - 1, :], 0.0)
            nc.sync.dma_start(
                out=sdh_sb[:, : NKC - 1, :],
                in_=sdh_d[: 128 * (NKC - 1), :].rearrange("(c p) e -> p c e", p=128))
            nc.sync.dma_start(out=sdh_sb[: 128 - KT, NKC - 1, :],
                              in_=sdh_d[128 * (NKC - 1):, :])

            sdf_sb = ph1.tile([128, NVC, 3 * NB], f32)
            nc.gpsimd.dma_start(
                out=sdf_sb[:, : NVC - 1, :],
                in_=sdf_d[: 128 * (NVC - 1), :].rearrange("(c p) e -> p c e", p=128))
            nc.gpsimd.dma_start(out=sdf_sb[: 128 - VT, NVC - 1, :],
                                in_=sdf_d[128 * (NVC - 1):, :])
            vtf_sb = ph1.tile([128, NVC, 3], f32)
            nc.gpsimd.dma_start(
                out=vtf_sb[:, : NVC - 1, :],
                in_=vtf_d[: 128 * (NVC - 1), :].rearrange("(c p) e -> p c e", p=128))
            nc.gpsimd.dma_start(out=vtf_sb[: 128 - VT, NVC - 1, :],
                                in_=vtf_d[128 * (NVC - 1):, :])

            lbsh_sb = ph1.tile([128, NLC, J], f32)
            nc.vector.memset(lbsh_sb[:, NLC - 1, :], 0.0)
            nc.gpsimd.dma_start(
                out=lbsh_sb[:, : NLC - 1, :],
                in_=lbs_d[: 128 * (NLC - 1), :].rearrange("(c p) e -> p c e", p=128))
            nc.gpsimd.dma_start(out=lbsh_sb[: 128 - LT, NLC - 1, :],
                                in_=lbs_d[128 * (NLC - 1):, :])

            # ===== JregT via packed PE transposes =====
            jregT = ph1.tile([128, NVC * J], f32)        # (v128, c*24+j)
            for g in range(_cdiv(NVC, 4)):
                cs = list(range(g * 4, min(g * 4 + 4, NVC)))
                ps = ps1.tile([128, 4 * J], f32, tag="ps_tr")
                for i, c in enumerate(cs):
                    w = 128 if c < NVC - 1 else 128 - VT
                    nc.tensor.transpose(ps[:w, i * J:(i + 1) * J],
                                        jreg_sb[:, c * 128: c * 128 + w],
                                        ident[:J, :J])
                nc.scalar.copy(out=jregT[:, g * 4 * J: (g * 4 + len(cs)) * J],
                               in_=ps[:, : len(cs) * J])

            # ===== JSD accumulation over full V =====
            jsd_ps = psacc.tile([J, 3 * NB], f32, tag="jsd")
            jvt_ps = psacc.tile([J, 3], f32, tag="jvt")
            for c in range(NVC):
                w = 128 if c < NVC - 1 else 128 - VT
                kw = dict(start=(c == 0), stop=(c == NVC - 1))
                lt = jregT[:w, c * J:(c + 1) * J]
                nc.tensor.matmul(jsd_ps, lt, sdf_sb[:w, c, :], **kw)
                nc.tensor.matmul(jvt_ps, lt, vtf_sb[:w, c, :], **kw)
            jsd_sb = sm.tile([J, 3 * NB], f32)
            nc.scalar.copy(out=jsd_sb, in_=jsd_ps)
            jvt_sb = sm.tile([J, 3], f32)
            nc.scalar.copy(out=jvt_sb, in_=jvt_ps)

            # jsdr (11, (k,j)): rows 0..9 JSD_l, row 10 Jvt
            jsdr = sm.tile([NB + 1, 3 * J], f32)
            ps_j = ps1.tile([NB, 3 * J], f32, tag="ps_tr")
            for k in range(3):
                nc.tensor.transpose(ps_j[:, k * J:(k + 1) * J],
                                    jsd_sb[:, k * NB:(k + 1) * NB], ident[:J, :J])
            nc.scalar.copy(out=jsdr[:NB, :], in_=ps_j)
            ps_v = ps1.tile([3, J], f32, tag="ps_tr2")
            nc.tensor.transpose(ps_v, jvt_sb, ident[:J, :J])
            jvtT = sm.tile([3, J], f32)
            nc.scalar.copy(out=jvtT, in_=ps_v)
            for k in range(3):
                nc.sync.dma_start(out=jsdr[NB:NB + 1, k * J:(k + 1) * J],
                                  in_=jvtT[k:k + 1, :])

            # ===== betasT (+ones row), Jloc =====
            betasT = sm.tile([NB + 1, BG], f32)
            nc.vector.memset(betasT, 1.0)
            ps_b = ps1.tile([NB, BG], f32, tag="ps_tr")
            nc.tensor.transpose(ps_b, betas_sb, ident[:BG, :BG])
            nc.scalar.copy(out=betasT[:NB, :], in_=ps_b)

            jloc_ps = psjl.tile([BG, 3 * J], f32, tag="ps_jl")
            nc.tensor.matmul(jloc_ps, betasT, jsdr, start=True, stop=True)
            jloc = sm.tile([BG, 3 * J], f32)
            nc.scalar.copy(out=jloc, in_=jloc_ps)

            # ===== Rodrigues =====
            pose3 = pose_sb.rearrange("p (j k) -> p k j", k=3)
            bias0 = sm.tile([BG, 1], f32)
            nc.vector.memset(bias0, 0.0)
            biasq = sm.tile([BG, 1], f32)
            nc.vector.memset(biasq, float(np.pi / 2))
            peps = sm.tile([BG, J * 3], f32)
            nc.vector.tensor_scalar_add(peps, pose_sb, 1e-8)
            sq = sm.tile([BG, J * 3], f32)
            nc.vector.tensor_mul(sq, peps, peps)
            sq3 = sq.rearrange("p (j k) -> p k j", k=3)
            ang = sm.tile([BG, J], f32)
            nc.vector.tensor_add(ang, sq3[:, 0, :], sq3[:, 1, :])
            nc.vector.tensor_add(ang, ang, sq3[:, 2, :])
            nc.scalar.activation(out=ang, in_=ang, func=Act.Sqrt, bias=bias0)
            inv = sm.tile([BG, J], f32)
            nc.vector.reciprocal(out=inv, in_=ang)
            sin = sm.tile([BG, J], f32)
            nc.scalar.activation(out=sin, in_=ang, func=Act.Sin, bias=bias0)
            omc = sm.tile([BG, J], f32)          # 1 - cos(ang)
            nc.scalar.activation(out=omc, in_=ang, func=Act.Sin, bias=biasq)
            nc.vector.tensor_scalar(omc, omc, -1.0, 1.0, Alu.mult, Alu.add)

            rd = sm.tile([BG, 3, J], f32)
            for k in range(3):
                nc.vector.tensor_mul(rd[:, k, :], pose3[:, k, :], inv)
            pr = sm.tile([BG, 9, J], f32)   # xx yy zz xy xz yz sx sy sz
            for i, (a, b) in enumerate([(0, 0), (1, 1), (2, 2),
                                        (0, 1), (0, 2), (1, 2)]):
                nc.vector.tensor_mul(pr[:, i, :], rd[:, a, :], rd[:, b, :])
            for k in range(3):
                nc.vector.tensor_mul(pr[:, 6 + k, :], sin, rd[:, k, :])
            XX, YY, ZZ, XY, XZ, YZ, SX, SY, SZ = range(9)

            R = sm.tile([BG, 9, J], f32)    # (r*3+c, j), diag includes +1
            t9 = sm.tile([BG, 3, J], f32)   # omc*(sum of other squares)
            for r, (a, b) in enumerate([(YY, ZZ), (XX, ZZ), (XX, YY)]):
                nc.vector.tensor_add(t9[:, r, :], pr[:, a, :], pr[:, b, :])
                nc.vector.tensor_mul(t9[:, r, :], t9[:, r, :], omc)
                nc.vector.tensor_scalar(R[:, r * 3 + r, :], t9[:, r, :],
                                        -1.0, 1.0, Alu.mult, Alu.add)
            m9 = sm.tile([BG, 3, J], f32)
            for i, pidx in enumerate([XY, XZ, YZ]):
                nc.vector.tensor_mul(m9[:, i, :], pr[:, pidx, :], omc)
            for (rc, mi, si, op) in [(1, 0, SZ, Alu.subtract), (3, 0, SZ, Alu.add),
                                     (2, 1, SY, Alu.add), (6, 1, SY, Alu.subtract),
                                     (5, 2, SX, Alu.subtract), (7, 2, SX, Alu.add)]:
                nc.vector.tensor_tensor(R[:, rc, :], m9[:, mi, :], pr[:, si, :], op)

            # ===== pose_feature -> pfT =====
            pf = sm.tile([BG, J - 1, 9], f32)
            for rc in range(9):
                if rc in (0, 4, 8):
                    nc.vector.tensor_scalar_mul(pf[:, :, rc],
                                                t9[:, rc // 3, 1:], -1.0)
                else:
                    nc.vector.tensor_copy(out=pf[:, :, rc], in_=R[:, rc, 1:])
            pff = pf.rearrange("p a b -> p (a b)")
            pfT0 = sm.tile([128, BG], bf16)
            ps_p = ps1.tile([128, BG], f32, tag="ps_tr")
            nc.tensor.transpose(ps_p, pff[:, :128], ident)
            nc.scalar.copy(out=pfT0, in_=ps_p)
            pfT1 = sm.tile([P - 128, BG], bf16)
            ps_q = ps1.tile([P - 128, BG], f32, tag="ps_tr2")
            nc.tensor.transpose(ps_q, pff[:, 128:P], ident)
            nc.scalar.copy(out=pfT1, in_=ps_q)

            # ===== rel, Tm, kinematic chain =====
            j3 = jloc.rearrange("p (k j) -> p k j", k=3)
            rel = sm.tile([BG, 3, J], f32)
            nc.vector.tensor_copy(out=rel[:, :, 0], in_=j3[:, :, 0])
            for (j0, cnt, p0, pstep) in _parent_runs(par):
                if pstep == 1:
                    src = j3[:, :, p0:p0 + cnt]
                else:
                    src = bcast_j(j3, p0, cnt)
                nc.vector.tensor_sub(rel[:, :, j0:j0 + cnt],
                                     j3[:, :, j0:j0 + cnt], src)

            Tm = sm.tile([BG, J, 12], f32)
            Tm4 = Tm.rearrange("p j (r c) -> p j r c", c=4)
            nc.vector.tensor_copy(out=Tm4[:, :, :, :3],
                                  in_=R.rearrange("p (r c) j -> p j r c", c=3))
            nc.vector.tensor_copy(out=Tm4[:, :, :, 3],
                                  in_=rel.rearrange("p k j -> p j k"))

            W = sm.tile([BG, J, 12], f32)
            W4 = W.rearrange("p j (r c) -> p j r c", c=4)
            nc.vector.tensor_copy(out=W[:, 0, :], in_=Tm[:, 0, :])
            for j in range(1, J):
                pj = int(par[j])
                for r in range(3):
                    nc.vector.tensor_scalar_mul(W4[:, j, r, :], Tm4[:, j, 0, :],
                                                W4[:, pj, r, 0:1])
                    for m in (1, 2):
                        nc.vector.scalar_tensor_tensor(
                            W4[:, j, r, :], Tm4[:, j, m, :], W4[:, pj, r, m:m + 1],
                            W4[:, j, r, :], Alu.mult, Alu.add)
                    nc.vector.tensor_scalar_add(W4[:, j, r, 3:4], W4[:, j, r, 3:4],
                                                W4[:, pj, r, 3:4])

            # transforms / posed_joints outputs
            nc.sync.dma_start(out=tf_o[:, :, :3, :],
                              in_=W.rearrange("p j (r c) -> p j r c", c=4))
            bot = sm.tile([BG, J, 4], f32)
            nc.vector.memset(bot, 0.0)
            nc.vector.memset(bot[:, :, 3], 1.0)
            nc.sync.dma_start(out=tf_o[:, :, 3, :], in_=bot)
            pjc = sm.tile([BG, J, 3], f32)
            nc.vector.tensor_copy(out=pjc, in_=W4[:, :, :, 3])
            nc.sync.dma_start(out=pj_o, in_=pjc)

            # ===== A = W - [0 | W @ [Jloc;0]] (gpsimd) =====
            Am = sm.tile([BG, J, 12], f32)
            Am4 = Am.rearrange("p j (r c) -> p j r c", c=4)
            nc.gpsimd.tensor_copy(out=Am4[:, :, :, :3], in_=W4[:, :, :, :3])
            acc3 = sm.tile([BG, 3, J], f32)
            t3 = sm.tile([BG, J], f32)
            for r in range(3):
                nc.gpsimd.tensor_mul(acc3[:, r, :], W4[:, :, r, 0], j3[:, 0, :])
                for k in (1, 2):
                    nc.gpsimd.tensor_mul(t3, W4[:, :, r, k], j3[:, k, :])
                    nc.gpsimd.tensor_add(acc3[:, r, :], acc3[:, r, :], t3)
                nc.gpsimd.tensor_sub(Am4[:, :, r, 3], W4[:, :, r, 3], acc3[:, r, :])
            nc.sync.dma_start(out=A_o[:, :, :3, :],
                              in_=Am.rearrange("p j (r c) -> p j r c", c=4))
            nc.sync.dma_start(out=A_o[:, :, 3, :], in_=bot)

            # ===== SDT build =====
            sdt_stage = ph1.tile([PK * NB, NGK * 128], f32)
            sdh_f = sdh_sb.rearrange("p c e -> p (c e)")
            for g in range(NGK):
                c0 = g * PK
                nch = min(PK, NKC - c0)
                ps_s = ps1.tile([PK * NB, 128], f32, tag="ps_tr")
                nc.tensor.transpose(ps_s[: nch * NB, :],
                                    sdh_f[:, c0 * NB:(c0 + nch) * NB], ident)
                nc.scalar.copy(out=sdt_stage[: nch * NB, g * 128:(g + 1) * 128],
                               in_=ps_s[: nch * NB, :])
            sdt4 = sdt.rearrange("l (g c p) -> l c g p", c=PK, p=128)
            for c in range(PK):
                gcnt = len([g for g in range(NGK) if g * PK + c < NKC])
                nc.sync.dma_start(
                    out=sdt4[:NB, c, :gcnt, :],
                    in_=sdt_stage[c * NB:(c + 1) * NB, : gcnt * 128]
                        .rearrange("l (g p) -> l g p", p=128))
            nc.sync.dma_start(out=sdt[NB:NB + 1, :VKH], in_=vth_d[None, :])

            # ===== lbsT build =====
            lbst_stage = ph1.tile([PL * J, NGL * 128], f32)
            lbsh_f = lbsh_sb.rearrange("p c e -> p (c e)")
            for g in range(NGL):
                c0 = g * PL
                ncl = min(PL, NLC - c0)
                ps_l = ps1.tile([PL * J, 128], f32, tag="ps_tr")
                nc.tensor.transpose(ps_l[: ncl * J, :],
                                    lbsh_f[:, c0 * J:(c0 + ncl) * J], ident)
                nc.scalar.copy(out=lbst_stage[: ncl * J, g * 128:(g + 1) * 128],
                               in_=ps_l[: ncl * J, :])
            lbsT4 = lbsT.rearrange("j (g c p) -> j c g p", c=PL, p=128)
            for c in range(PL):
                gcnt = len([g for g in range(NGL) if g * PL + c < NLC])
                nc.gpsimd.dma_start(
                    out=lbsT4[:, c, :gcnt, :],
                    in_=lbst_stage[c * J:(c + 1) * J, : gcnt * 128]
                        .rearrange("j (g p) -> j g p", p=128))

        # ---------------- v_posed stream ----------------
        with tc.tile_pool(name="pdst", bufs=3) as pdst, \
             tc.tile_pool(name="psvp", bufs=2, space="PSUM") as psvp:
            for t in range(NTN):
                n0 = t * NT
                nn = min(NT, VKH - n0)
                pd0 = pdst.tile([128, NT], bf16, tag="pd0")
                nc.sync.dma_start(out=pd0[:, :nn], in_=pd_d[:128, n0:n0 + nn])
                pd1 = pdst.tile([P - 128, NT], bf16, tag="pd1")
                nc.sync.dma_start(out=pd1[:, :nn], in_=pd_d[128:, n0:n0 + nn])
                ps_vp = psvp.tile([BG, NT], f32, tag="ps_vp")
                nc.tensor.matmul(ps_vp[:, :nn], betasT, sdt[:, n0:n0 + nn],
                                 start=True, stop=False)
                nc.tensor.matmul(ps_vp[:, :nn], pfT0, pd0[:, :nn],
                                 start=False, stop=False)
                nc.tensor.matmul(ps_vp[:, :nn], pfT1, pd1[:, :nn],
                                 start=False, stop=True)
                nc.scalar.copy(out=vp_sb[:, n0:n0 + nn], in_=ps_vp[:, :nn])
                nc.sync.dma_start(
                    out=vposed_o.rearrange("b v k -> b (v k)")[:, n0:n0 + nn],
                    in_=vp_sb[:, n0:n0 + nn])
            # A_T (24, 12, 128) transposes, emitted after vp so the PE
            # queue is not stalled on the serial DVE chain
            for g in range(3):
                ps_a = psvp.tile([J, 4 * BG], f32, tag="ps_a")
                for i in range(4):
                    nc.tensor.transpose(ps_a[:, i * BG:(i + 1) * BG],
                                        Am[:, :, g * 4 + i], ident[:BG, :BG])
                nc.scalar.copy(
                    out=AT[:, g * 4:(g + 1) * 4, :].rearrange("j a b -> j (a b)"),
                    in_=ps_a)

        # ---------------- skinning ----------------
        vp3 = vp_sb.rearrange("p (v k) -> p k v", k=3)
        with tc.tile_pool(name="vout", bufs=2) as vout, \
             tc.tile_pool(name="mt", bufs=2) as mt, \
             tc.tile_pool(name="psT", bufs=2, space="PSUM") as psT:
            for t in range(NTV):
                v0 = t * NT
                nv = min(NT, VH - v0)
                vo = vout.tile([BG, NT, 3], f32, tag="vo")
                for p in range(3):
                    ps_T = psT.tile([BG, 4, NT], f32, tag="ps_T")
                    for k in range(4):
                        nc.tensor.matmul(ps_T[:, k, :nv], AT[:, p * 4 + k, :],
                                         lbsT[:, v0:v0 + nv], start=True, stop=True)
                    m0 = mt.tile([BG, NT], f32, tag="m0")
                    m1 = mt.tile([BG, NT], f32, tag="m1")
                    t3sb = mt.tile([BG, NT], f32, tag="t3sb")
                    nc.vector.tensor_mul(m0[:, :nv], ps_T[:, 0, :nv],
                                         vp3[:, 0, v0:v0 + nv])
                    nc.vector.tensor_mul(m1[:, :nv], ps_T[:, 1, :nv],
                                         vp3[:, 1, v0:v0 + nv])
                    nc.scalar.copy(out=t3sb[:, :nv], in_=ps_T[:, 3, :nv])
                    nc.gpsimd.tensor_add(m0[:, :nv], m0[:, :nv], m1[:, :nv])
                    nc.vector.tensor_mul(m1[:, :nv], ps_T[:, 2, :nv],
                                         vp3[:, 2, v0:v0 + nv])
                    nc.gpsimd.tensor_add(m0[:, :nv], m0[:, :nv], t3sb[:, :nv])
                    nc.vector.tensor_add(vo[:, :nv, p], m0[:, :nv], m1[:, :nv])
                nc.sync.dma_start(out=verts_o[:, v0:v0 + nv, :],
                                  in_=vo[:, :nv, :])

    nc.compile()
    return nc


def _get_nc(par_key):
    if par_key not in _CACHE:
        _CACHE[par_key] = _build(np.asarray(par_key, dtype=np.int32))
    return _CACHE[par_key]


def kernel(betas, pose, v_template, shapedirs, posedirs, J_regressor,
           lbs_weights, parents):
    from concourse.bass_utils import run_bass_kernel_spmd

    par = tuple(int(x) for x in np.asarray(parents))
    nc = _get_nc(par)

    f = np.ascontiguousarray
    sd_flat = np.asarray(shapedirs, np.float32).reshape(V * 3, NB)
    sd_full = np.asarray(shapedirs, np.float32).reshape(V, 3 * NB)
    vt_flat = np.asarray(v_template, np.float32).reshape(V * 3)
    import ml_dtypes
    pd = np.asarray(posedirs, np.float32).astype(ml_dtypes.bfloat16)
    jreg = np.asarray(J_regressor, np.float32)
    lbs = np.asarray(lbs_weights, np.float32)

    in_maps = []
    for c in range(NCORES):
        bg, vh = c // 2, c % 2
        in_maps.append({
            "betas_g": f(np.asarray(betas, np.float32)[bg * BG:(bg + 1) * BG]),
            "pose_g": f(np.asarray(pose, np.float32)[bg * BG:(bg + 1) * BG]),
            "pd_half": f(pd[:, vh * VKH:(vh + 1) * VKH]),
            "sd_half": f(sd_flat[vh * VKH:(vh + 1) * VKH]),
            "sd_full": f(sd_full),
            "vt_half": f(vt_flat[vh * VKH:(vh + 1) * VKH]),
            "vt_full": f(np.asarray(v_template, np.float32)),
            "jreg": f(jreg),
            "lbs_half": f(lbs[vh * VH:(vh + 1) * VH]),
        })

    import os as _os
    _r = run_bass_kernel_spmd(
        nc, in_maps, core_ids=list(range(NCORES)),
        trace=bool(_os.environ.get("BM_TRACE")))
    global LAST_RESULTS
    LAST_RESULTS = _r
    res = _r.results

    verts = np.empty((B, V, 3), np.float32)
    vposed = np.empty((B, V, 3), np.float32)
    A = np.empty((B, J, 4, 4), np.float32)
    tf = np.empty((B, J, 4, 4), np.float32)
    pj = np.empty((B, J, 3), np.float32)
    for c in range(NCORES):
        bg, vh = c // 2, c % 2
        bs = slice(bg * BG, (bg + 1) * BG)
        vs = slice(vh * VH, (vh + 1) * VH)
        verts[bs, vs] = res[c]["verts"]
        vposed[bs, vs] = res[c]["vposed"]
        if vh == 0:
            A[bs] = res[c]["A_out"]
            tf[bs] = res[c]["tf_out"]
            pj[bs] = res[c]["pj_out"]
    return verts, pj, A, tf, vposed
